# revision 36
# baseline (speedup 1.0000x reference)
"""EnvironmentalContextAttention on 8 trn2 NeuronCores.

Model (reference.py):
    q,k,v = heads(x@Wq+bq), heads(x@Wk+bk), heads(x@Wv+bv)      # [B,H,S,HD]
    scores = (q @ k^T) / sqrt(HD) * gate[b,h]                   # [B,H,S,S]
    gate   = sigmoid((env@We+be)@Wm+bm)                         # [B,H]
    out    = (softmax(scores) @ v).merge_heads() @ Wo + bo      # [B,S,D]

Sharding: 8 cores = 2 batches x 4 head-groups (4 heads each). Each core
computes its heads' attention and a partial out-projection (transposed,
[D, S]); the host sums the 4 partials per batch and re-transposes.

Device-side simplifications (exact, not approximations):
  * bk drops: a per-query constant shift in scores cancels in softmax.
  * bq folds into the exp bias; gate/sqrt(HD) folds into the exp scale.
  * bv, bo: softmax rows sum to 1, so the host adds bv@Wo + bo once.
  * no running-max shift: |gated scores| < ~12, exp can't overflow fp32.
  * sum-of-exp rides as a ones row appended to each head's V tile.

Perf structure:
  * Scores have contraction K=64, so a head pair lives at partitions
    0-63 / 64-127 and its matmuls are issued adjacently: disjoint PE
    row-groups run concurrently (measured 443 -> 192 ns per 512-col mm).
  * exp splits across engines: even head exact ScalarE exp; odd head a
    Schraudolph bit-trick on VectorE (i16 = round(s*128/ln2*g + const),
    bitcast to bf16 ~ exp, ~3% sawtooth; net output err ~8.5e-3).
  * Software pipeline: iteration i's score matmuls interleave with
    iteration i-1's ctx matmuls per key-chunk, so the exp engines hide
    behind ctx PE time and PSUM slots recycle without stalling PE.
  * 1/sumexp: custom-DVE reciprocal_approx_fast on a partition-0 copy of
    the sum row (the custom op misbehaves on nonzero base partitions);
    broadcast on GpSimd, multiply on VectorE.
  * out-projection runs per s-half right after that half's ctn is ready,
    PSUM pools scoped per block; outT is bf16 to halve the output DMA.
"""

import contextlib

import ml_dtypes
import numpy as np

import concourse.bass as bass
import concourse.mybir as mybir
from concourse import bacc
from concourse.tile import TileContext
from concourse import bass_utils

# problem constants (hardcoded per contract)
B, S, D, H, E = 2, 2048, 1024, 16, 256
HD = D // H            # 64
N_CORES = 8
HPC = H // 4           # 4 heads per core
J = HPC * HD           # 256 local columns
P = 128
KC = D // P            # 8 contraction chunks
TC = S // P            # 16 key chunks
JC = J // P            # 2 local j chunks (also: head pairs per core)
NC_O = D // P          # 8 output row chunks
SC = S // 512          # 4 query chunks of 512

F32 = mybir.dt.float32
BF16 = mybir.dt.bfloat16
I16 = mybir.dt.int16

A_SCH = 128.0 / np.log(2.0)          # Schraudolph slope for bf16 bit grid
B_SCH = 128.0 * (127.0 - 0.043)      # centered-sawtooth intercept


def build_nc(repeats: int = 1, tiny_out: bool = False, upto='out'):
    STAGES = ['p1', 'pA', 'pB', 'norm', 'out']
    LVL = STAGES.index(upto)
    nc = bacc.Bacc("TRN2", target_bir_lowering=False, debug=False,
                   num_devices=N_CORES)

    xT = nc.dram_tensor("xT", [D, S], BF16, kind="ExternalInput").ap()
    wq = nc.dram_tensor("wq", [D, J], BF16, kind="ExternalInput").ap()
    wk = nc.dram_tensor("wk", [D, J], BF16, kind="ExternalInput").ap()
    wv = nc.dram_tensor("wv", [D, HPC * (HD + 1)], BF16, kind="ExternalInput").ap()
    wo = nc.dram_tensor("wo", [J, D], BF16, kind="ExternalInput").ap()
    gates = nc.dram_tensor("gates", [1, HPC], F32, kind="ExternalInput").ap()
    gdve = nc.dram_tensor("gdve", [1, HPC], F32, kind="ExternalInput").ap()
    cb = nc.dram_tensor("cb", [S, HPC], F32, kind="ExternalInput").ap()
    cbd = nc.dram_tensor("cbd", [S, HPC], F32, kind="ExternalInput").ap()
    if tiny_out:
        outT = nc.dram_tensor("outT", [D, S], BF16, kind="Internal").ap()
        tiny = nc.dram_tensor("tiny", [P, 512], F32, kind="ExternalOutput").ap()
    else:
        outT = nc.dram_tensor("outT", [D, S], BF16, kind="ExternalOutput").ap()
        tiny = None

    with TileContext(nc) as tc:
        with (
            tc.tile_pool(name="const", bufs=1) as const_pool,
            tc.tile_pool(name="xw", bufs=1) as xw_pool,
            tc.tile_pool(name="qkv", bufs=1) as qkv_pool,
            tc.tile_pool(name="outsb", bufs=6) as out_pool,
        ):
            # ---- constants / inputs ----
            gates1 = const_pool.tile([1, HPC], F32)
            nc.sync.dma_start(out=gates1[:], in_=gates[:])
            gates_sb = const_pool.tile([P, HPC], F32)
            nc.gpsimd.partition_broadcast(gates_sb[:], gates1[:])
            gdve1 = const_pool.tile([1, HPC], F32)
            nc.sync.dma_start(out=gdve1[:], in_=gdve[:])
            gdve_sb = const_pool.tile([P, HPC], F32)
            nc.gpsimd.partition_broadcast(gdve_sb[:], gdve1[:])

            cb_sb = const_pool.tile([P, TC, HPC], F32)
            nc.sync.dma_start(out=cb_sb[:], in_=cb.rearrange("(c p) h -> p c h", p=P))
            cbd_sb = const_pool.tile([P, TC, HPC], F32)
            nc.sync.dma_start(out=cbd_sb[:], in_=cbd.rearrange("(c p) h -> p c h", p=P))

            wo_sb = xw_pool.tile([P, JC, D], BF16)
            nc.sync.dma_start(out=wo_sb[:], in_=wo.rearrange("(c p) n -> p c n", p=P))

            # input tiles live across loop iterations so the reload for the
            # next repeat can be prefetched mid-body (hidden under attention)
            xv_cm = tc.tile_pool(name="xv", bufs=1)
            xv_pool = xv_cm.__enter__()
            x_sb = xv_pool.tile([P, KC, S], BF16, name="x_sb")
            wv_sb = xv_pool.tile([P, KC, HPC * (HD + 1)], BF16, name="wv_sb")
            wq_sb = xv_pool.tile([P, KC, J], BF16, name="wq_sb")
            wk_sb = xv_pool.tile([P, KC, J], BF16, name="wk_sb")
            xTr = xT.rearrange("(c p) s -> p c s", p=P)
            wvr = wv.rearrange("(c p) j -> p c j", p=P)
            wqr = wq.rearrange("(c p) j -> p c j", p=P)
            wkr = wk.rearrange("(c p) j -> p c j", p=P)

            def emit_input_dmas():
                for k in range(KC):
                    nc.sync.dma_start(out=wq_sb[:, k], in_=wqr[:, k])
                for k in range(KC):
                    nc.sync.dma_start(out=x_sb[:, k], in_=xTr[:, k])
                for k in range(KC):
                    nc.sync.dma_start(out=wk_sb[:, k], in_=wkr[:, k])
                for k in range(KC):
                    nc.sync.dma_start(out=wv_sb[:, k], in_=wvr[:, k])

            prefetch = repeats > 1
            if prefetch:
                emit_input_dmas()  # prologue load; body reloads mid-iteration

            rep_cm = (tc.For_i(0, repeats, 1) if repeats > 1
                      else contextlib.nullcontext())
            with rep_cm:
                # ---- phase 1: projections (baseline-shaped loop) ----
                qt_sb = qkv_pool.tile([P, JC, S], BF16, tag="qt", name="qt_sb")
                kt_sb = qkv_pool.tile([P, JC, S], BF16, tag="kt", name="kt_sb")
                v_sb = qkv_pool.tile([P, TC, HPC, HD + 1], BF16, tag="v", name="v_sb")
                ctn = qkv_pool.tile([P, JC, S], BF16, tag="ctn", name="ctn")

                with (
                    tc.tile_pool(name="ps_proj", bufs=4, space="PSUM") as ps_proj,
                ):
                    if not prefetch:
                        emit_input_dmas()
                    for w_sb, dst in ((wq_sb, qt_sb), (wk_sb, kt_sb)):
                        for jc in range(JC):
                            for sc in range(SC):
                                ps = ps_proj.tile([P, 512], F32, tag="pj", name="pj")
                                for k in range(KC):
                                    nc.tensor.matmul(
                                        ps[:],
                                        lhsT=w_sb[:, k, jc * P:(jc + 1) * P],
                                        rhs=x_sb[:, k, sc * 512:(sc + 1) * 512],
                                        start=(k == 0), stop=(k == KC - 1),
                                    )
                                nc.vector.tensor_copy(
                                    out=dst[:, jc, sc * 512:(sc + 1) * 512], in_=ps[:])

                # ---- attention, software-pipelined per s-half block ----
                def pass_a(ps_sc, expA_pool, expD_pool, hj, half):
                    """Emit pair-concurrent score mms + split exp for chunk t
                    of (hj, half); returns (ets0, ets1) accumulated lists."""
                    h0, h1 = 2 * hj, 2 * hj + 1
                    ets0, ets1 = [], []

                    def emit_t(t):
                        psa = ps_sc.tile([P, 1024], F32, tag="sc", name="sca")
                        psb = ps_sc.tile([P, 1024], F32, tag="sc", name="scb")
                        for i in range(2):
                            scq = 2 * half + i
                            nc.tensor.matmul(
                                psa[:, i * 512:(i + 1) * 512],
                                lhsT=kt_sb[0:HD, hj, t * P:(t + 1) * P],
                                rhs=qt_sb[0:HD, hj, scq * 512:(scq + 1) * 512],
                                start=True, stop=True,
                            )
                            nc.tensor.matmul(
                                psb[:, i * 512:(i + 1) * 512],
                                lhsT=kt_sb[HD:P, hj, t * P:(t + 1) * P],
                                rhs=qt_sb[HD:P, hj, scq * 512:(scq + 1) * 512],
                                start=True, stop=True,
                            )
                        et0 = expA_pool.tile([P, 1024], BF16, tag="exp", name="et0")
                        nc.scalar.activation(
                            et0[:], psa[:], mybir.ActivationFunctionType.Exp,
                            bias=cb_sb[:, t, h0:h0 + 1],
                            scale=gates_sb[:, h0:h0 + 1],
                        )
                        et1 = expD_pool.tile([P, 1024], I16, tag="expd", name="et1")
                        nc.vector.tensor_scalar(
                            out=et1[:], in0=psb[:],
                            scalar1=gdve_sb[:, h1:h1 + 1],
                            scalar2=cbd_sb[:, t, h1:h1 + 1],
                            op0=mybir.AluOpType.mult,
                            op1=mybir.AluOpType.add,
                        )
                        ets0.append(et0)
                        ets1.append(et1)

                    return ets0, ets1, emit_t

                def pass_b_t(ctxs, hj, t, ets0, ets1):
                    """ctx mms of key-chunk t for iteration (hj): 4 matmuls."""
                    h0, h1 = 2 * hj, 2 * hj + 1
                    e1bf = ets1[t][:].bitcast(BF16)
                    for i in range(2):
                        nc.tensor.matmul(
                            ctxs[0][i][:],
                            lhsT=v_sb[:, t, h0, :],
                            rhs=ets0[t][:, i * 512:(i + 1) * 512],
                            start=(t == 0), stop=(t == TC - 1),
                        )
                        nc.tensor.matmul(
                            ctxs[1][i][:],
                            lhsT=v_sb[:, t, h1, :],
                            rhs=e1bf[:, i * 512:(i + 1) * 512],
                            start=(t == 0), stop=(t == TC - 1),
                        )

                def norm(ctx_pool, work_pool, ctxs, hj, half):
                    """Normalize both heads of finished iteration (hj, half)."""
                    if LVL < 3:
                        return
                    for hp in range(2):
                        h = 2 * hj + hp
                        hpp = 64 * (h % 2)
                        cs = ctx_pool.tile([HD, 1024], F32, tag="cs", name="cs")
                        srow = work_pool.tile([1, 1024], F32, tag="sr",
                                              name="srow", bufs=2)
                        for i in range(2):
                            nc.scalar.copy(out=cs[:, i * 512:(i + 1) * 512],
                                           in_=ctxs[hp][i][0:HD, :])
                            nc.scalar.copy(out=srow[:, i * 512:(i + 1) * 512],
                                           in_=ctxs[hp][i][HD:HD + 1, :])
                        rr = work_pool.tile([1, 1024], F32, tag="rr",
                                            name="rr", bufs=2)
                        nc.vector.reciprocal_approx_fast(rr[:], srow[:])
                        pb = work_pool.tile([HD, 1024], F32, tag="pb",
                                            name="pb", bufs=2)
                        nc.gpsimd.partition_broadcast(pb[:], rr[:])
                        nc.vector.tensor_tensor(
                            out=ctn[hpp:hpp + HD, hj,
                                    half * 1024:(half + 1) * 1024],
                            in0=cs[:], in1=pb[:],
                            op=mybir.AluOpType.mult)

                def out_chunk(ps_pool, n, half, copy_eng):
                    """Two out-proj mms + copies + one 1024-wide DMA for row
                    block n of s-half `half`."""
                    ot = out_pool.tile([P, 1024], BF16, tag="ot",
                                       name="ot", bufs=6)
                    po = ps_pool.tile([P, 1024], F32, tag="sc", name="po")
                    for i in range(2):
                        scq = 2 * half + i
                        for jc in range(JC):
                            nc.tensor.matmul(
                                po[:, i * 512:(i + 1) * 512],
                                lhsT=wo_sb[:, jc, n * P:(n + 1) * P],
                                rhs=ctn[:, jc, scq * 512:(scq + 1) * 512],
                                start=(jc == 0), stop=(jc == JC - 1),
                            )
                        if (copy_eng + i) % 2 == 0:
                            nc.vector.tensor_copy(
                                out=ot[:, i * 512:(i + 1) * 512],
                                in_=po[:, i * 512:(i + 1) * 512])
                        else:
                            nc.scalar.copy(
                                out=ot[:, i * 512:(i + 1) * 512],
                                in_=po[:, i * 512:(i + 1) * 512])
                    nc.sync.dma_start(
                        out=outT[n * P:(n + 1) * P,
                                 half * 1024:(half + 1) * 1024],
                        in_=ot[:])

                iters = [(half, hj) for half in range(2) for hj in range(JC)]
                if LVL < 1:
                    iters = []
                with (
                    tc.tile_pool(name="expA", bufs=18) as expA_pool,
                    tc.tile_pool(name="expD", bufs=18) as expD_pool,
                    tc.tile_pool(name="ctxsb", bufs=4) as ctx_pool,
                    tc.tile_pool(name="work", bufs=8) as work_pool,
                    tc.tile_pool(name="ps_sc", bufs=2, space="PSUM") as ps_sc,
                ):
                    prev = None
                    if iters:
                        # iteration 0: scores interleave with the V projection
                        # (fills the exp-gated prologue with PE work)
                        half, hj = iters[0]
                        ets0, ets1, emit_t = pass_a(
                            ps_sc, expA_pool, expD_pool, hj, half)
                        with tc.tile_pool(name="ps_v", bufs=2,
                                          space="PSUM") as ps_v:
                            for t in range(TC):
                                emit_t(t)
                                psv = ps_v.tile([P, HPC * (HD + 1)], F32,
                                                tag="pv", name="psv")
                                for k in range(KC):
                                    nc.tensor.matmul(
                                        psv[:],
                                        lhsT=x_sb[:, k, t * P:(t + 1) * P],
                                        rhs=wv_sb[:, k, :],
                                        start=(k == 0), stop=(k == KC - 1),
                                    )
                                nc.vector.tensor_copy(out=v_sb[:, t], in_=psv[:])
                                nc.vector.memset(v_sb[:, t, :, HD:HD + 1], 1.0)
                        prev = (hj, ets0, ets1, half)
                        if prefetch:
                            # x/weights are dead now: reload for the next
                            # repeat, hidden under the remaining iterations
                            emit_input_dmas()
                    with tc.tile_pool(name="ps_ctx", bufs=4,
                                      space="PSUM") as ps_ctx:
                        for half, hj in iters[1:]:
                            ets0, ets1, emit_t = pass_a(
                                ps_sc, expA_pool, expD_pool, hj, half)
                            if LVL >= 2:
                                ctxs = [[ps_ctx.tile([HD + 1, 512], F32,
                                                     tag="ctx", name="ctx")
                                         for _ in range(2)] for _ in range(2)]
                            for t in range(TC):
                                emit_t(t)
                                if LVL >= 2:
                                    pass_b_t(ctxs, prev[0], t, prev[1], prev[2])
                            if LVL >= 2:
                                norm(ctx_pool, work_pool, ctxs, prev[0], prev[3])
                            prev = (hj, ets0, ets1, half)
                        # epilogue: ctx for the last pair, with half-0
                        # out-projection chunks interleaved (ctn half 0 done);
                        # po tiles borrow the now-idle ps_sc ring.
                        if LVL >= 2 and prev is not None:
                            ctxs = [[ps_ctx.tile([HD + 1, 512], F32,
                                                 tag="ctx", name="ctx")
                                     for _ in range(2)] for _ in range(2)]
                            for t in range(TC):
                                pass_b_t(ctxs, prev[0], t, prev[1], prev[2])
                                if LVL >= 4 and t % 2 == 1:
                                    out_chunk(ps_sc, t // 2, 0, t // 2)
                            norm(ctx_pool, work_pool, ctxs, prev[0], prev[3])
                            if LVL >= 4:
                                for n in range(NC_O):
                                    out_chunk(ps_sc, n, 1, n)
            xv_cm.__exit__(None, None, None)

            if tiny_out:
                tt = out_pool.tile([P, 512], F32, name="tt", tag="tt", bufs=1)
                nc.vector.memset(tt[:], 1.0)
                nc.sync.dma_start(out=tiny[:], in_=tt[:])

    nc.compile()
    return nc


_NC_CACHE = {}


def get_nc(repeats: int = 1, tiny_out: bool = False):
    key = (repeats, tiny_out)
    if key not in _NC_CACHE:
        _NC_CACHE[key] = build_nc(repeats, tiny_out)
    return _NC_CACHE[key]


def host_prep(inputs):
    """Shard + precompute per-core input maps; return (in_maps, out_bias_row)."""
    f = {k: np.asarray(v, dtype=np.float64) for k, v in inputs.items()}
    x, env = f["x"], f["env_context"]
    Wq, Wk, Wv, Wo = f["Wq"], f["Wk"], f["Wv"], f["Wo"]
    bq, bk, bv, bo = f["bq"], f["bk"], f["bv"], f["bo"]
    We, be, Wm, bm = f["We"], f["be"], f["Wm"], f["bm"]

    gate = 1.0 / (1.0 + np.exp(-((env @ We + be) @ Wm + bm)))  # [B, H]
    scale = gate / np.sqrt(HD)                                  # [B, H]

    in_maps = []
    for c in range(N_CORES):
        b, g = divmod(c, 4)
        cols = slice(J * g, J * (g + 1))
        wv_pad = np.zeros((D, HPC * (HD + 1)), np.float64)
        cbm = np.zeros((S, HPC), np.float64)
        for i in range(HPC):
            h = HPC * g + i
            hc = slice(HD * h, HD * (h + 1))
            wv_pad[:, i * (HD + 1):i * (HD + 1) + HD] = Wv[:, hc]
            # bq.k_t row: x[b] @ (Wk_h @ bq_h), pre-scaled by gate/sqrt(HD)
            cbm[:, i] = scale[b, h] * (x[b] @ (Wk[:, hc] @ bq[hc]))
        in_maps.append({
            "xT": np.ascontiguousarray(x[b].T).astype(ml_dtypes.bfloat16),
            "wq": np.ascontiguousarray(Wq[:, cols]).astype(ml_dtypes.bfloat16),
            "wk": np.ascontiguousarray(Wk[:, cols]).astype(ml_dtypes.bfloat16),
            "wv": wv_pad.astype(np.float32).astype(ml_dtypes.bfloat16),
            "wo": np.ascontiguousarray(Wo[J * g:J * (g + 1), :]).astype(ml_dtypes.bfloat16),
            "gates": scale[b, HPC * g:HPC * (g + 1)].reshape(1, HPC).astype(np.float32),
            "gdve": (A_SCH * scale[b, HPC * g:HPC * (g + 1)]).reshape(1, HPC).astype(np.float32),
            "cb": cbm.astype(np.float32),
            "cbd": (A_SCH * cbm + B_SCH).astype(np.float32),
        })
    out_bias_row = (bv @ Wo + bo).astype(np.float32)  # [D]
    return in_maps, out_bias_row


def assemble(results, out_bias_row):
    out = np.zeros((B, S, D), np.float32)
    for c in range(N_CORES):
        b = c // 4
        out[b] += results[c]["outT"].astype(np.float32).T
    out += out_bias_row[None, None, :]
    return out


def kernel(**inputs):
    import time as _time

    nc = get_nc(1)
    in_maps, out_bias_row = host_prep(inputs)
    last_err = None
    for _attempt in range(4):
        try:
            res = bass_utils.run_bass_kernel_spmd(
                nc, in_maps, core_ids=list(range(N_CORES)))
            return assemble(res.results, out_bias_row)
        except Exception as e:  # transient NRT/axon hiccups recover on retry
            last_err = e
            _time.sleep(5)
    raise last_err


# revision 37
# speedup vs baseline: 1.0567x; 1.0567x over previous
"""EnvironmentalContextAttention on 8 trn2 NeuronCores.

Model (reference.py):
    q,k,v = heads(x@Wq+bq), heads(x@Wk+bk), heads(x@Wv+bv)      # [B,H,S,HD]
    scores = (q @ k^T) / sqrt(HD) * gate[b,h]                   # [B,H,S,S]
    gate   = sigmoid((env@We+be)@Wm+bm)                         # [B,H]
    out    = (softmax(scores) @ v).merge_heads() @ Wo + bo      # [B,S,D]

Sharding: 8 cores = 2 batches x 4 head-groups (4 heads each). Each core
computes its heads' attention and a partial out-projection (transposed,
[D, S]); the host sums the 4 partials per batch and re-transposes.

Device-side simplifications (exact, not approximations):
  * bk drops: a per-query constant shift in scores cancels in softmax.
  * bq folds into the exp bias; gate/sqrt(HD) folds into the exp scale.
  * bv, bo: softmax rows sum to 1, so the host adds bv@Wo + bo once.
  * no running-max shift: |gated scores| < ~12, exp can't overflow fp32.
  * sum-of-exp rides as a ones row appended to each head's V tile.

Perf structure:
  * Scores have contraction K=64, so a head pair lives at partitions
    0-63 / 64-127 and its matmuls are issued adjacently: disjoint PE
    row-groups run concurrently (measured 443 -> 192 ns per 512-col mm).
  * exp splits across engines: even head exact ScalarE exp; odd head a
    Schraudolph bit-trick on VectorE (i16 = round(s*128/ln2*g + const),
    bitcast to bf16 ~ exp, ~3% sawtooth; net output err ~8.5e-3).
  * Software pipeline: iteration i's score matmuls interleave with
    iteration i-1's ctx matmuls per key-chunk, so the exp engines hide
    behind ctx PE time and PSUM slots recycle without stalling PE.
  * 1/sumexp: custom-DVE reciprocal_approx_fast on a partition-0 copy of
    the sum row (the custom op misbehaves on nonzero base partitions);
    broadcast on GpSimd, multiply on VectorE.
  * out-projection runs per s-half right after that half's ctn is ready,
    PSUM pools scoped per block; outT is bf16 to halve the output DMA.
"""

import contextlib

import ml_dtypes
import numpy as np

import concourse.bass as bass
import concourse.mybir as mybir
from concourse import bacc
from concourse.tile import TileContext
from concourse import bass_utils

# problem constants (hardcoded per contract)
B, S, D, H, E = 2, 2048, 1024, 16, 256
HD = D // H            # 64
N_CORES = 8
HPC = H // 4           # 4 heads per core
J = HPC * HD           # 256 local columns
P = 128
KC = D // P            # 8 contraction chunks
TC = S // P            # 16 key chunks
JC = J // P            # 2 local j chunks (also: head pairs per core)
NC_O = D // P          # 8 output row chunks
SC = S // 512          # 4 query chunks of 512

F32 = mybir.dt.float32
BF16 = mybir.dt.bfloat16
I16 = mybir.dt.int16

A_SCH = 128.0 / np.log(2.0)          # Schraudolph slope for bf16 bit grid
B_SCH = 128.0 * (127.0 - 0.043)      # centered-sawtooth intercept


def build_nc(repeats: int = 1, tiny_out: bool = False, upto='out'):
    STAGES = ['p1', 'pA', 'pB', 'norm', 'out']
    LVL = STAGES.index(upto)
    nc = bacc.Bacc("TRN2", target_bir_lowering=False, debug=False,
                   num_devices=N_CORES)

    xT = nc.dram_tensor("xT", [D, S], BF16, kind="ExternalInput").ap()
    wq = nc.dram_tensor("wq", [D, J], BF16, kind="ExternalInput").ap()
    wk = nc.dram_tensor("wk", [D, J], BF16, kind="ExternalInput").ap()
    wv = nc.dram_tensor("wv", [D, HPC * (HD + 1)], BF16, kind="ExternalInput").ap()
    wo = nc.dram_tensor("wo", [J, D], BF16, kind="ExternalInput").ap()
    gates = nc.dram_tensor("gates", [1, HPC], F32, kind="ExternalInput").ap()
    gdve = nc.dram_tensor("gdve", [1, HPC], F32, kind="ExternalInput").ap()
    cb = nc.dram_tensor("cb", [S, HPC], F32, kind="ExternalInput").ap()
    cbd = nc.dram_tensor("cbd", [S, HPC], F32, kind="ExternalInput").ap()
    if tiny_out:
        outT = nc.dram_tensor("outT", [D, S], BF16, kind="Internal").ap()
        tiny = nc.dram_tensor("tiny", [P, 512], F32, kind="ExternalOutput").ap()
    else:
        outT = nc.dram_tensor("outT", [D, S], BF16, kind="ExternalOutput").ap()
        tiny = None

    with TileContext(nc) as tc:
        with (
            tc.tile_pool(name="const", bufs=1) as const_pool,
            tc.tile_pool(name="xw", bufs=1) as xw_pool,
            tc.tile_pool(name="qkv", bufs=1) as qkv_pool,
            tc.tile_pool(name="outsb", bufs=6) as out_pool,
        ):
            # ---- constants / inputs ----
            gates1 = const_pool.tile([1, HPC], F32)
            nc.sync.dma_start(out=gates1[:], in_=gates[:])
            gates_sb = const_pool.tile([P, HPC], F32)
            nc.gpsimd.partition_broadcast(gates_sb[:], gates1[:])
            gdve1 = const_pool.tile([1, HPC], F32)
            nc.sync.dma_start(out=gdve1[:], in_=gdve[:])
            gdve_sb = const_pool.tile([P, HPC], F32)
            nc.gpsimd.partition_broadcast(gdve_sb[:], gdve1[:])

            cb_sb = const_pool.tile([P, TC, HPC], F32)
            nc.sync.dma_start(out=cb_sb[:], in_=cb.rearrange("(c p) h -> p c h", p=P))
            cbd_sb = const_pool.tile([P, TC, HPC], F32)
            nc.sync.dma_start(out=cbd_sb[:], in_=cbd.rearrange("(c p) h -> p c h", p=P))

            wo_sb = xw_pool.tile([P, JC, D], BF16)
            nc.sync.dma_start(out=wo_sb[:], in_=wo.rearrange("(c p) n -> p c n", p=P))

            # input tiles live across loop iterations so the reload for the
            # next repeat can be prefetched mid-body (hidden under attention)
            xv_cm = tc.tile_pool(name="xv", bufs=1)
            xv_pool = xv_cm.__enter__()
            x_sb = xv_pool.tile([P, KC, S], BF16, name="x_sb")
            wv_sb = xv_pool.tile([P, KC, HPC * (HD + 1)], BF16, name="wv_sb")
            wq_sb = xv_pool.tile([P, KC, J], BF16, name="wq_sb")
            wk_sb = xv_pool.tile([P, KC, J], BF16, name="wk_sb")
            xTr = xT.rearrange("(c p) s -> p c s", p=P)
            wvr = wv.rearrange("(c p) j -> p c j", p=P)
            wqr = wq.rearrange("(c p) j -> p c j", p=P)
            wkr = wk.rearrange("(c p) j -> p c j", p=P)

            def emit_input_dmas():
                for k in range(KC):
                    nc.sync.dma_start(out=wq_sb[:, k], in_=wqr[:, k])
                for k in range(KC):
                    nc.sync.dma_start(out=x_sb[:, k], in_=xTr[:, k])
                for k in range(KC):
                    nc.sync.dma_start(out=wk_sb[:, k], in_=wkr[:, k])
                for k in range(KC):
                    nc.sync.dma_start(out=wv_sb[:, k], in_=wvr[:, k])

            # Mid-body reload prefetch measured WORSE (~+17us) than paying
            # the input-DMA ramp at the body top — keep the classic order.
            prefetch = False
            if prefetch:
                emit_input_dmas()  # prologue load; body reloads mid-iteration

            rep_cm = (tc.For_i(0, repeats, 1) if repeats > 1
                      else contextlib.nullcontext())
            with rep_cm:
                # ---- phase 1: projections (baseline-shaped loop) ----
                qt_sb = qkv_pool.tile([P, JC, S], BF16, tag="qt", name="qt_sb")
                kt_sb = qkv_pool.tile([P, JC, S], BF16, tag="kt", name="kt_sb")
                v_sb = qkv_pool.tile([P, TC, HPC, HD + 1], BF16, tag="v", name="v_sb")
                ctn = qkv_pool.tile([P, JC, S], BF16, tag="ctn", name="ctn")

                with (
                    tc.tile_pool(name="ps_proj", bufs=4, space="PSUM") as ps_proj,
                ):
                    if not prefetch:
                        emit_input_dmas()
                    for w_sb, dst in ((wq_sb, qt_sb), (wk_sb, kt_sb)):
                        for jc in range(JC):
                            for sc in range(SC):
                                ps = ps_proj.tile([P, 512], F32, tag="pj", name="pj")
                                for k in range(KC):
                                    nc.tensor.matmul(
                                        ps[:],
                                        lhsT=w_sb[:, k, jc * P:(jc + 1) * P],
                                        rhs=x_sb[:, k, sc * 512:(sc + 1) * 512],
                                        start=(k == 0), stop=(k == KC - 1),
                                    )
                                nc.vector.tensor_copy(
                                    out=dst[:, jc, sc * 512:(sc + 1) * 512], in_=ps[:])

                # ---- attention, software-pipelined per s-half block ----
                def pass_a(ps_sc, expA_pool, expD_pool, hj, half):
                    """Emit pair-concurrent score mms + split exp for chunk t
                    of (hj, half); returns (ets0, ets1) accumulated lists."""
                    h0, h1 = 2 * hj, 2 * hj + 1
                    ets0, ets1 = [], []

                    def emit_t(t):
                        psa = ps_sc.tile([P, 1024], F32, tag="sc", name="sca")
                        psb = ps_sc.tile([P, 1024], F32, tag="sc", name="scb")
                        for i in range(2):
                            scq = 2 * half + i
                            nc.tensor.matmul(
                                psa[:, i * 512:(i + 1) * 512],
                                lhsT=kt_sb[0:HD, hj, t * P:(t + 1) * P],
                                rhs=qt_sb[0:HD, hj, scq * 512:(scq + 1) * 512],
                                start=True, stop=True,
                            )
                            nc.tensor.matmul(
                                psb[:, i * 512:(i + 1) * 512],
                                lhsT=kt_sb[HD:P, hj, t * P:(t + 1) * P],
                                rhs=qt_sb[HD:P, hj, scq * 512:(scq + 1) * 512],
                                start=True, stop=True,
                            )
                        et0 = expA_pool.tile([P, 1024], BF16, tag="exp", name="et0")
                        nc.scalar.activation(
                            et0[:], psa[:], mybir.ActivationFunctionType.Exp,
                            bias=cb_sb[:, t, h0:h0 + 1],
                            scale=gates_sb[:, h0:h0 + 1],
                        )
                        et1 = expD_pool.tile([P, 1024], I16, tag="expd", name="et1")
                        nc.vector.tensor_scalar(
                            out=et1[:], in0=psb[:],
                            scalar1=gdve_sb[:, h1:h1 + 1],
                            scalar2=cbd_sb[:, t, h1:h1 + 1],
                            op0=mybir.AluOpType.mult,
                            op1=mybir.AluOpType.add,
                        )
                        ets0.append(et0)
                        ets1.append(et1)

                    return ets0, ets1, emit_t

                def pass_b_t(ctxs, hj, t, ets0, ets1):
                    """ctx mms of key-chunk t for iteration (hj): 4 matmuls."""
                    h0, h1 = 2 * hj, 2 * hj + 1
                    e1bf = ets1[t][:].bitcast(BF16)
                    for i in range(2):
                        nc.tensor.matmul(
                            ctxs[0][i][:],
                            lhsT=v_sb[:, t, h0, :],
                            rhs=ets0[t][:, i * 512:(i + 1) * 512],
                            start=(t == 0), stop=(t == TC - 1),
                        )
                        nc.tensor.matmul(
                            ctxs[1][i][:],
                            lhsT=v_sb[:, t, h1, :],
                            rhs=e1bf[:, i * 512:(i + 1) * 512],
                            start=(t == 0), stop=(t == TC - 1),
                        )

                def norm(ctx_pool, work_pool, ctxs, hj, half):
                    """Normalize both heads of finished iteration (hj, half)."""
                    if LVL < 3:
                        return
                    for hp in range(2):
                        h = 2 * hj + hp
                        hpp = 64 * (h % 2)
                        cs = ctx_pool.tile([HD, 1024], F32, tag="cs", name="cs")
                        srow = work_pool.tile([1, 1024], F32, tag="sr",
                                              name="srow", bufs=2)
                        for i in range(2):
                            nc.scalar.copy(out=cs[:, i * 512:(i + 1) * 512],
                                           in_=ctxs[hp][i][0:HD, :])
                            nc.scalar.copy(out=srow[:, i * 512:(i + 1) * 512],
                                           in_=ctxs[hp][i][HD:HD + 1, :])
                        rr = work_pool.tile([1, 1024], F32, tag="rr",
                                            name="rr", bufs=2)
                        nc.vector.reciprocal_approx_fast(rr[:], srow[:])
                        pb = work_pool.tile([HD, 1024], F32, tag="pb",
                                            name="pb", bufs=2)
                        nc.gpsimd.partition_broadcast(pb[:], rr[:])
                        nc.vector.tensor_tensor(
                            out=ctn[hpp:hpp + HD, hj,
                                    half * 1024:(half + 1) * 1024],
                            in0=cs[:], in1=pb[:],
                            op=mybir.AluOpType.mult)

                def out_chunk(ps_pool, n, half, copy_eng):
                    """Two out-proj mms + copies + one 1024-wide DMA for row
                    block n of s-half `half`."""
                    ot = out_pool.tile([P, 1024], BF16, tag="ot",
                                       name="ot", bufs=6)
                    po = ps_pool.tile([P, 1024], F32, tag="sc", name="po")
                    for i in range(2):
                        scq = 2 * half + i
                        for jc in range(JC):
                            nc.tensor.matmul(
                                po[:, i * 512:(i + 1) * 512],
                                lhsT=wo_sb[:, jc, n * P:(n + 1) * P],
                                rhs=ctn[:, jc, scq * 512:(scq + 1) * 512],
                                start=(jc == 0), stop=(jc == JC - 1),
                            )
                        if (copy_eng + i) % 2 == 0:
                            nc.vector.tensor_copy(
                                out=ot[:, i * 512:(i + 1) * 512],
                                in_=po[:, i * 512:(i + 1) * 512])
                        else:
                            nc.scalar.copy(
                                out=ot[:, i * 512:(i + 1) * 512],
                                in_=po[:, i * 512:(i + 1) * 512])
                    nc.sync.dma_start(
                        out=outT[n * P:(n + 1) * P,
                                 half * 1024:(half + 1) * 1024],
                        in_=ot[:])

                iters = [(half, hj) for half in range(2) for hj in range(JC)]
                if LVL < 1:
                    iters = []
                with (
                    tc.tile_pool(name="expA", bufs=18) as expA_pool,
                    tc.tile_pool(name="expD", bufs=18) as expD_pool,
                    tc.tile_pool(name="ctxsb", bufs=4) as ctx_pool,
                    tc.tile_pool(name="work", bufs=8) as work_pool,
                    tc.tile_pool(name="ps_sc", bufs=2, space="PSUM") as ps_sc,
                ):
                    prev = None
                    if iters:
                        # iteration 0: scores interleave with the V projection
                        # (fills the exp-gated prologue with PE work)
                        half, hj = iters[0]
                        ets0, ets1, emit_t = pass_a(
                            ps_sc, expA_pool, expD_pool, hj, half)
                        with tc.tile_pool(name="ps_v", bufs=2,
                                          space="PSUM") as ps_v:
                            for t in range(TC):
                                emit_t(t)
                                psv = ps_v.tile([P, HPC * (HD + 1)], F32,
                                                tag="pv", name="psv")
                                for k in range(KC):
                                    nc.tensor.matmul(
                                        psv[:],
                                        lhsT=x_sb[:, k, t * P:(t + 1) * P],
                                        rhs=wv_sb[:, k, :],
                                        start=(k == 0), stop=(k == KC - 1),
                                    )
                                nc.vector.tensor_copy(out=v_sb[:, t], in_=psv[:])
                                nc.vector.memset(v_sb[:, t, :, HD:HD + 1], 1.0)
                        prev = (hj, ets0, ets1, half)
                        if prefetch:
                            # x/weights are dead now: reload for the next
                            # repeat, hidden under the remaining iterations
                            emit_input_dmas()
                    with tc.tile_pool(name="ps_ctx", bufs=4,
                                      space="PSUM") as ps_ctx:
                        for half, hj in iters[1:]:
                            ets0, ets1, emit_t = pass_a(
                                ps_sc, expA_pool, expD_pool, hj, half)
                            if LVL >= 2:
                                ctxs = [[ps_ctx.tile([HD + 1, 512], F32,
                                                     tag="ctx", name="ctx")
                                         for _ in range(2)] for _ in range(2)]
                            for t in range(TC):
                                emit_t(t)
                                if LVL >= 2:
                                    pass_b_t(ctxs, prev[0], t, prev[1], prev[2])
                            if LVL >= 2:
                                norm(ctx_pool, work_pool, ctxs, prev[0], prev[3])
                            prev = (hj, ets0, ets1, half)
                        # epilogue: ctx for the last pair, with half-0
                        # out-projection chunks interleaved (ctn half 0 done);
                        # po tiles borrow the now-idle ps_sc ring.
                        if LVL >= 2 and prev is not None:
                            ctxs = [[ps_ctx.tile([HD + 1, 512], F32,
                                                 tag="ctx", name="ctx")
                                     for _ in range(2)] for _ in range(2)]
                            for t in range(TC):
                                pass_b_t(ctxs, prev[0], t, prev[1], prev[2])
                                if LVL >= 4 and t % 2 == 1:
                                    out_chunk(ps_sc, t // 2, 0, t // 2)
                            norm(ctx_pool, work_pool, ctxs, prev[0], prev[3])
                            if LVL >= 4:
                                for n in range(NC_O):
                                    out_chunk(ps_sc, n, 1, n)
            xv_cm.__exit__(None, None, None)

            if tiny_out:
                tt = out_pool.tile([P, 512], F32, name="tt", tag="tt", bufs=1)
                nc.vector.memset(tt[:], 1.0)
                nc.sync.dma_start(out=tiny[:], in_=tt[:])

    nc.compile()
    return nc


_NC_CACHE = {}


def get_nc(repeats: int = 1, tiny_out: bool = False):
    key = (repeats, tiny_out)
    if key not in _NC_CACHE:
        _NC_CACHE[key] = build_nc(repeats, tiny_out)
    return _NC_CACHE[key]


def host_prep(inputs):
    """Shard + precompute per-core input maps; return (in_maps, out_bias_row)."""
    f = {k: np.asarray(v, dtype=np.float64) for k, v in inputs.items()}
    x, env = f["x"], f["env_context"]
    Wq, Wk, Wv, Wo = f["Wq"], f["Wk"], f["Wv"], f["Wo"]
    bq, bk, bv, bo = f["bq"], f["bk"], f["bv"], f["bo"]
    We, be, Wm, bm = f["We"], f["be"], f["Wm"], f["bm"]

    gate = 1.0 / (1.0 + np.exp(-((env @ We + be) @ Wm + bm)))  # [B, H]
    scale = gate / np.sqrt(HD)                                  # [B, H]

    in_maps = []
    for c in range(N_CORES):
        b, g = divmod(c, 4)
        cols = slice(J * g, J * (g + 1))
        wv_pad = np.zeros((D, HPC * (HD + 1)), np.float64)
        cbm = np.zeros((S, HPC), np.float64)
        for i in range(HPC):
            h = HPC * g + i
            hc = slice(HD * h, HD * (h + 1))
            wv_pad[:, i * (HD + 1):i * (HD + 1) + HD] = Wv[:, hc]
            # bq.k_t row: x[b] @ (Wk_h @ bq_h), pre-scaled by gate/sqrt(HD)
            cbm[:, i] = scale[b, h] * (x[b] @ (Wk[:, hc] @ bq[hc]))
        in_maps.append({
            "xT": np.ascontiguousarray(x[b].T).astype(ml_dtypes.bfloat16),
            "wq": np.ascontiguousarray(Wq[:, cols]).astype(ml_dtypes.bfloat16),
            "wk": np.ascontiguousarray(Wk[:, cols]).astype(ml_dtypes.bfloat16),
            "wv": wv_pad.astype(np.float32).astype(ml_dtypes.bfloat16),
            "wo": np.ascontiguousarray(Wo[J * g:J * (g + 1), :]).astype(ml_dtypes.bfloat16),
            "gates": scale[b, HPC * g:HPC * (g + 1)].reshape(1, HPC).astype(np.float32),
            "gdve": (A_SCH * scale[b, HPC * g:HPC * (g + 1)]).reshape(1, HPC).astype(np.float32),
            "cb": cbm.astype(np.float32),
            "cbd": (A_SCH * cbm + B_SCH).astype(np.float32),
        })
    out_bias_row = (bv @ Wo + bo).astype(np.float32)  # [D]
    return in_maps, out_bias_row


def assemble(results, out_bias_row):
    out = np.zeros((B, S, D), np.float32)
    for c in range(N_CORES):
        b = c // 4
        out[b] += results[c]["outT"].astype(np.float32).T
    out += out_bias_row[None, None, :]
    return out


def kernel(**inputs):
    import time as _time

    nc = get_nc(1)
    in_maps, out_bias_row = host_prep(inputs)
    last_err = None
    for _attempt in range(4):
        try:
            res = bass_utils.run_bass_kernel_spmd(
                nc, in_maps, core_ids=list(range(N_CORES)))
            return assemble(res.results, out_bias_row)
        except Exception as e:  # transient NRT/axon hiccups recover on retry
            last_err = e
            _time.sleep(5)
    raise last_err
